# revision 16
# baseline (speedup 1.0000x reference)
"""Multi-head attention on 8 TRN2 NeuronCores — pipelined-attention build.

Sharding: core c handles batch b = c // 2 and head-half hh = c % 2
(8 of 16 heads). Each core computes a partial out^T; the host sums the
two partials per batch, adds the bias and transposes back.

Pipeline: attention is a stream of 16 chunks (4 head-pairs x 4
q-chunks), processed in HALF-chunK units of 8 k-steps. Scores+exp for
half-chunk h+1 are emitted interleaved with the ctx matmuls of
half-chunk h, so the PE never depends on a just-issued exp: ctx
consumes P^T tiles exp'd a full 8 k-steps earlier, and ACT has 8
k-steps of lookahead. The attention phase is ACT-bound on HW (~1 us
per [128,1024] exp); everything else hides under it. V-projection
fills the pipeline-fill slot (interleaved with chunk 0's scores); the
out-projection nch0 groups fill the drain slot.

Per-core layout (feature-major, no on-chip transposes):
  xT [1024, 2048] bf16 host-pretransposed; QT/KT per pair [128, 2048];
  V natural [s, dh]+ones cols (ctx matmul also yields softmax denom);
  scores^T psum [128 s_k, 1024 = 2 heads x 512 q]; exp on ACT;
  denominator row: reciprocal (DVE) + partition_broadcast (POOL);
  normalize muls deferred one chunk; out^T = wo^T-slices @ ct.
"""

import numpy as np
import ml_dtypes

import concourse.bacc as bacc
import concourse.tile as tile
import concourse.mybir as mybir
from concourse.bass_utils import run_bass_kernel_spmd

bf16 = ml_dtypes.bfloat16
FP32 = mybir.dt.float32
BF16 = mybir.dt.bfloat16
EXP = mybir.ActivationFunctionType.Exp

B, S, DL = 4, 2048, 1024
H, DH = 16, 64          # global heads
NH = 8                  # heads per core
HD = NH * DH            # 512 feature cols per core
NPAIR = NH // 2         # 4 head pairs
KT = DL // 128          # 8 k-tiles over d_latent
ST = S // 128           # 16 tiles over sequence
QCH = 1024              # q-chunk (2 heads x 512 q)
NQC = S // QCH
NCH = 16                # chunks = NPAIR * 4
SCALE = 1.0 / np.sqrt(DH)

N_CORES = 8


def _build(reps: int = 1, loop: int = 0, ablate=()):
    nc = bacc.Bacc(None, target_bir_lowering=False)

    xT = nc.dram_tensor("xT", [DL, S], BF16, kind="ExternalInput")
    wq = nc.dram_tensor("wq", [DL, HD], BF16, kind="ExternalInput")
    wk = nc.dram_tensor("wk", [DL, HD], BF16, kind="ExternalInput")
    wv = nc.dram_tensor("wv", [DL, HD], BF16, kind="ExternalInput")
    wo = nc.dram_tensor("wo", [HD, DL], BF16, kind="ExternalInput")
    out = nc.dram_tensor("out", [DL, S], BF16, kind="ExternalOutput")

    with tile.TileContext(nc) as tc:
        with (
            tc.tile_pool(name="persist", bufs=1) as pp,
            tc.tile_pool(name="pt", bufs=24) as ptp,
            tc.tile_pool(name="small", bufs=2) as smp,
            tc.tile_pool(name="outsb", bufs=3) as osp,
            tc.tile_pool(name="psA", bufs=2, space="PSUM") as psA,
            tc.tile_pool(name="psB", bufs=2, space="PSUM") as psB,
        ):
            if loop:
                with tc.For_i(0, loop, 1):
                    _body(nc, tc, pp, ptp, smp, osp, psA, psB,
                          xT, wq, wk, wv, wo, out, ablate)
            else:
                for _ in range(reps):
                    _body(nc, tc, pp, ptp, smp, osp, psA, psB,
                          xT, wq, wk, wv, wo, out, ablate)
    nc.compile()
    return nc


def _body(nc, tc, pp, ptp, smp, osp, psA, psB, xT, wq, wk, wv, wo, out, ablate=()):
    # ---- persistent tiles (tag-keyed; reused across reps) ----
    xt = [pp.tile([128, S], BF16, tag=f"xt{k}", name=f"xt{k}") for k in range(KT)]
    wq_sb = [pp.tile([128, HD], BF16, tag=f"wq{k}", name=f"wq{k}") for k in range(KT)]
    wk_sb = [pp.tile([128, HD], BF16, tag=f"wk{k}", name=f"wk{k}") for k in range(KT)]
    wv_sb = [pp.tile([128, HD], BF16, tag=f"wv{k}", name=f"wv{k}") for k in range(KT)]
    wo_sb = [pp.tile([128, DL], BF16, tag=f"wo{t}", name=f"wo{t}") for t in range(NPAIR)]
    qt = [pp.tile([128, S], BF16, tag=f"qt{p}", name=f"qt{p}") for p in range(NPAIR)]
    kt_ = [pp.tile([128, S], BF16, tag=f"kt{p}", name=f"kt{p}") for p in range(NPAIR)]
    vt = [pp.tile([128, NH * (DH + 4)], BF16, tag=f"vt{m}", name=f"vt{m}") for m in range(ST)]
    ct = [pp.tile([128, S], BF16, tag=f"ct{t}", name=f"ct{t}") for t in range(NPAIR)]
    if 'exp' in ablate:
        ptdummy = pp.tile([128, QCH], BF16, tag="ptdummy", name="ptdummy")
        nc.vector.memset(ptdummy[:, :], 0.001)

    # ---- input DMAs (xt first: V proj is the first consumer) ----
    for k in range(KT):
        nc.sync.dma_start(xt[k][:, :], xT[k * 128:(k + 1) * 128, :])
    for k in range(KT):
        nc.sync.dma_start(wv_sb[k][:, :], wv[k * 128:(k + 1) * 128, :])
    for k in range(KT):
        nc.sync.dma_start(wq_sb[k][:, :], wq[k * 128:(k + 1) * 128, :])
        nc.sync.dma_start(wk_sb[k][:, :], wk[k * 128:(k + 1) * 128, :])
    for t in range(NPAIR):
        nc.sync.dma_start(wo_sb[t][:, :], wo[t * 128:(t + 1) * 128, :])

    # ---- helpers ----
    def scores_exp(c, ki):
        """Scores + exp for chunk c, k-tile ki. Returns the P^T tile."""
        p, qch = divmod(c, 4)
        qsl = slice(qch * 512, (qch + 1) * 512)
        ksl = slice(ki * 128, (ki + 1) * 128)
        sc = psA.tile([128, QCH], FP32, tag="sc", name="sc")
        nc.tensor.matmul(sc[:, 0:512], kt_[p][0:64, ksl],
                         qt[p][0:64, qsl], start=True, stop=True,
                         tile_position=(0, 0))
        nc.tensor.matmul(sc[:, 512:1024], kt_[p][64:128, ksl],
                         qt[p][64:128, qsl], start=True, stop=True,
                         tile_position=(64, 0))
        if 'exp' in ablate:
            return ptdummy
        pt1 = ptp.tile([128, QCH], BF16, tag="pt", name="pt1")
        nc.scalar.activation(pt1[:, :], sc[:, :], EXP, scale=SCALE)
        return pt1

    def vproj(m):
        ps = psA.tile([128, QCH], FP32, tag="sc", name="vps")
        for k in range(KT):
            nc.tensor.matmul(ps[:, 0:HD],
                             xt[k][:, m * 128:(m + 1) * 128],
                             wv_sb[k][:, :],
                             start=(k == 0), stop=(k == KT - 1))
        v3 = vt[m][:, :].rearrange("p (h c) -> p h c", c=DH + 4)
        nc.vector.tensor_copy(v3[:, :, 0:DH],
                              ps[:, 0:HD].rearrange("p (h c) -> p h c", h=NH))
        nc.vector.memset(v3[:, :, DH:DH + 4], 1.0)

    def proj_group(p_, dst, w_sb, nch):
        ps = psA.tile([128, QCH], FP32, tag="sc", name="proj")
        for half in range(2):
            nsl = slice(half * 512, half * 512 + 512)
            rsl = slice(nch * QCH + half * 512, nch * QCH + half * 512 + 512)
            for k in range(KT):
                nc.tensor.matmul(ps[:, nsl],
                                 w_sb[k][:, p_ * 128:(p_ + 1) * 128],
                                 xt[k][:, rsl],
                                 start=(k == 0), stop=(k == KT - 1))
        nc.vector.tensor_copy(dst[:, nch * QCH:(nch + 1) * QCH], ps[:, :])

    def proj_groups_for(p_):
        # q-nch0 / k-nch0 / k-nch1 must precede pair p_'s first chunk;
        # q-nch1 only precedes its chunk 2.
        if p_ >= NPAIR:
            return []
        return [(p_, qt[p_], wq_sb, 0), (p_, kt_[p_], wk_sb, 0),
                (p_, kt_[p_], wk_sb, 1), (p_, qt[p_], wq_sb, 1)]

    norm_pend = []

    def flush_norm():
        while norm_pend:
            p_, qsl_, rdst_ = norm_pend.pop(0)
            for hi_ in range(2):
                psl = slice(hi_ * 64, (hi_ + 1) * 64)
                csl = ct[p_][psl, qsl_]
                nc.vector.tensor_mul(csl, csl,
                                     rdst_[psl, hi_ * 512:hi_ * 512 + 512])

    def emit_ctx(c, ctxp, kj, ptj):
        p, _ = divmod(c, 4)
        for hi in range(2):
            lh = 2 * p + hi
            nc.tensor.matmul(ctxp[:, hi * 512:hi * 512 + 512],
                             vt[kj][:, lh * 68:lh * 68 + 68],
                             ptj[:, hi * 512:hi * 512 + 512],
                             start=(kj == 0), stop=(kj == ST - 1))

    def chunk_epilogue(c, ctxp):
        """Evacuate + start normalization for chunk c (psum ctxp)."""
        p, qch = divmod(c, 4)
        qsl = slice(qch * 512, (qch + 1) * 512)
        rsrc = smp.tile([1, QCH], BF16, tag="rsrc")
        rdst = smp.tile([128, QCH], BF16, tag="rdst")
        with nc.allow_low_precision(reason="softmax denom recip in bf16"):
            nc.vector.reciprocal(rsrc[0:1, :], ctxp[DH:DH + 1, :])
        nc.gpsimd.partition_broadcast(rdst[:, :], rsrc[0:1, :])
        nc.vector.tensor_copy(ct[p][0:64, qsl], ctxp[0:DH, 0:512])
        nc.vector.tensor_copy(ct[p][64:128, qsl], ctxp[0:DH, 512:1024])
        norm_pend.append((p, qsl, rdst))

    def out_group(mt, nch):
        msl = slice(mt * 128, (mt + 1) * 128)
        ob = osp.tile([128, QCH], BF16, tag="ob")
        ps = psA.tile([128, QCH], FP32, tag="sc", name="ops")
        for half in range(2):
            nsl = slice(half * 512, half * 512 + 512)
            rsl = slice(nch * QCH + half * 512, nch * QCH + half * 512 + 512)
            for t in range(NPAIR):
                nc.tensor.matmul(ps[:, nsl],
                                 wo_sb[t][:, msl],
                                 ct[t][:, rsl],
                                 start=(t == 0), stop=(t == NPAIR - 1))
        nc.vector.tensor_copy(ob[:, 0:512], ps[:, 0:512])
        nc.scalar.copy(ob[:, 512:1024], ps[:, 512:1024])
        nc.sync.dma_start(out[msl, nch * QCH:(nch + 1) * QCH], ob[:, :])

    # ---- prologue: pair-0 projections (first 3 groups) ----
    pending_proj = proj_groups_for(0)
    for _ in range(3):
        proj_group(*pending_proj.pop(0))

    # ---- fill slot: V projection interleaved with chunk-0 scores ----
    pts = {}  # (c, ki) -> P^T tile, consumed by the ctx one half-chunk later
    for m in range(ST):
        vproj(m)
        pts[(0, m)] = scores_exp(0, m)

    # ---- steady pipeline over half-chunks ----
    # Half-chunk unit h covers chunk c = h // 2, k-tiles [hk*8, hk*8+8).
    # Unit h runs ctx(h) interleaved with scores+exp(h+1).
    ctxp_cur = psB.tile([DH + 4, QCH], FP32, tag="ctx", name="ctxp")
    for h in range(2 * NCH):
        c, hk = divmod(h, 2)
        cn, hkn = divmod(h + 1, 2)
        if hk == 0 and c > 0 and c % 4 == 1:
            # schedule next pair's projections across this pair's slots
            pending_proj += proj_groups_for(c // 4 + 1)
        # one projection group at unit start: the PE has slack here and
        # the qt/kt copy lands well before its first scores reader
        if pending_proj:
            proj_group(*pending_proj.pop(0))
        for i in range(8):
            ki = hk * 8 + i
            if cn < NCH and cn != 0:
                kin = hkn * 8 + i
                pts[(cn, kin)] = scores_exp(cn, kin)
            emit_ctx(c, ctxp_cur, ki, pts.pop((c, ki)))
            if ki == 1:
                flush_norm()
        if hk == 1:
            chunk_epilogue(c, ctxp_cur)
            if c + 1 < NCH:
                ctxp_cur = psB.tile([DH + 4, QCH], FP32, tag="ctx", name="ctxp")
        elif c == NCH - 1:
            # drain slot: chunk 15's first half has no scores to pair
            # with; fill with the first out-projection groups (nch0).
            for mt in range(4):
                out_group(mt, 0)

    # final drain: chunk 15's second half also pairs with out groups
    for mt in range(4, KT):
        out_group(mt, 0)
    flush_norm()
    for mt in range(KT):
        out_group(mt, 1)


_NC_CACHE = {}


def _get_nc(reps: int = 1):
    if reps not in _NC_CACHE:
        _NC_CACHE[reps] = _build(reps)
    return _NC_CACHE[reps]


def shard_inputs(x, w_q, w_kv, w_out):
    """Full inputs -> per-core in_maps (host-side layout prep)."""
    ins = []
    for c in range(N_CORES):
        b, hh = c // 2, c % 2
        fsl = slice(hh * HD, (hh + 1) * HD)
        ins.append({
            "xT": np.ascontiguousarray(x[b].T).astype(bf16),
            "wq": np.ascontiguousarray(w_q[:, fsl]).astype(bf16),
            "wk": np.ascontiguousarray(w_kv[:, fsl]).astype(bf16),
            "wv": np.ascontiguousarray(w_kv[:, H * DH:][:, fsl]).astype(bf16),
            "wo": np.ascontiguousarray(w_out[fsl, :]).astype(bf16),
        })
    return ins


def unshard_output(results, b_out):
    out = np.empty((B, S, DL), np.float32)
    for b in range(B):
        acc = (results[2 * b]["out"].astype(np.float32)
               + results[2 * b + 1]["out"].astype(np.float32))  # [DL, S]
        out[b] = acc.T + b_out
    return out


def kernel(x, w_q, w_kv, w_out, b_out):
    nc = _get_nc()
    ins = shard_inputs(x, w_q, w_kv, w_out)
    res = run_bass_kernel_spmd(nc, ins, core_ids=list(range(N_CORES)))
    return unshard_output(res.results, b_out)
